# revision 14
# baseline (speedup 1.0000x reference)
"""CrossAttention Trainium2 SPMD kernel (v3).

Sharding: 8 cores = 2 batches x 4 head-groups (2 heads of 64 dims each).
Core i handles batch b=i//4, inner-dim slice [128*g:128*(g+1)], g=i%4.

Host pre-transposes x/context to D-major ([D, N] / [D, M]) and converts
them (and all weights) to bf16, halving input DMA and keeping every
matmul at 1 cycle/row.

Per-core pipeline:
  ctx phase (per 1024-token chunk): DMA ct^T / xt^T tiles; K^T psum =
    Wk^T ct^T (8 accum matmuls) -> KT sbuf bf16; V^T psum -> bf16 -> PE
    transpose -> token-major V_sb with a ones column per head (rowsum
    trick); Q^T psum -> QT_all sbuf bf16 (all queries projected up
    front so the attention loop has no projection work).
  attention (per 512-query chunk): per m-block of 128 keys, both heads'
    scores land in ONE [128,1024] psum tile (h0 cols 0:512, h1 cols
    512:1024) via two 64-contract matmuls; ONE 1024-wide exp on ACT ->
    u bf16. S/exp run 3 m-blocks ahead of AV (software pipeline) so the
    in-order PE never stalls on ACT. AV accumulates [V_h|1]^T u into
    psum [65, 512] over all 32 m-blocks. Softmax denominators:
    reciprocal on DVE, partition-broadcast via a ones[1,64] PE matmul
    (no DRAM round trip), normalize on DVE -> OT bf16. Out-proj uses
    1024-wide bf16 matmuls; the resulting y rows DMA out per 128-row
    block. Out-proj of chunk c is deferred into chunk c+1's stream to
    keep ACT busy across chunk boundaries.
Host sums the 4 partial Y per batch and adds the output bias.
"""
import numpy as np

import concourse.bass as bass
import concourse.tile as tile
from concourse import bacc, mybir
from concourse.bass_utils import run_bass_kernel_spmd
from concourse.masks import make_identity

F32 = mybir.dt.float32
F32R = mybir.dt.float32r
BF16 = mybir.dt.bfloat16
EXP = mybir.ActivationFunctionType.Exp

D = 1024          # model dim
DG = 128          # inner dims per core (2 heads x 64)
DH = 64           # head dim
SCALE = DH ** -0.5
N_CORES = 8


def build(N=4096, M=4096, nc_chunk=512):
    assert N % 1024 == 0 and M % 1024 == 0 and nc_chunk == 512
    nc = bacc.Bacc("TRN2", target_bir_lowering=False, debug=False,
                   num_devices=N_CORES)
    xt = nc.dram_tensor("xt", [D, N], BF16, kind="ExternalInput").ap()
    ct = nc.dram_tensor("ct", [D, M], BF16, kind="ExternalInput").ap()
    wq = nc.dram_tensor("wq", [D, DG], BF16, kind="ExternalInput").ap()
    wk = nc.dram_tensor("wk", [D, DG], BF16, kind="ExternalInput").ap()
    wv = nc.dram_tensor("wv", [D, DG], BF16, kind="ExternalInput").ap()
    wo = nc.dram_tensor("wo", [DG, D], BF16, kind="ExternalInput").ap()
    y = nc.dram_tensor("y", [N, D], F32, kind="ExternalOutput").ap()

    with tile.TileContext(nc) as tc:
        _kernel(tc, xt, ct, wq, wk, wv, wo, y, N, M)
    nc.compile()
    return nc


def _kernel(tc, xt, ct, wq, wk, wv, wo, y, N, M):
    nc = tc.nc
    NC = 512          # queries per attention chunk
    NT = M // 1024    # ctx-phase chunks
    MB = M // 128     # attention m-blocks
    CH = N // NC      # attention n-chunks
    P = 3             # S/exp software-pipeline depth (m-blocks)

    from contextlib import ExitStack
    with ExitStack() as ctx:
        consts = ctx.enter_context(tc.tile_pool(name="consts", bufs=1))
        big = ctx.enter_context(tc.tile_pool(name="big", bufs=1))
        cin = ctx.enter_context(tc.tile_pool(name="cin", bufs=2))
        xin = ctx.enter_context(tc.tile_pool(name="xin", bufs=2))
        vstage = ctx.enter_context(tc.tile_pool(name="vstage", bufs=2))
        upool = ctx.enter_context(tc.tile_pool(name="upool", bufs=P))
        rrp = ctx.enter_context(tc.tile_pool(name="rrp", bufs=4))
        otp = ctx.enter_context(tc.tile_pool(name="otp", bufs=4))
        ysb = ctx.enter_context(tc.tile_pool(name="ysb", bufs=3))

        # --- constants / weights ---
        ident = consts.tile([128, 128], F32)
        make_identity(nc, ident)
        identb = consts.tile([128, 128], BF16)
        nc.vector.tensor_copy(identb[:], ident[:])

        # ones row on partition 64 so the bcast matmul's lhsT/rhs partition
        # ranges line up with the reciprocal output (also on partition 64)
        ones_w = consts.tile([65, 64], F32R, name="ones_w")

        def load_w(ap, name):
            r = consts.tile([128, 8, 128], BF16, name=name)
            nc.sync.dma_start(out=r[:], in_=ap.rearrange("(kb p) c -> p kb c", p=128))
            return r

        wq_sb = load_w(wq, "wqb")
        wk_sb = load_w(wk, "wkb")
        wv_sb = load_w(wv, "wvb")

        wo_sb = consts.tile([64, 2, D], BF16, name="wo")
        nc.sync.dma_start(out=wo_sb[:], in_=wo.rearrange("(h p) d -> p h d", p=64))

        # persistent activations
        KT = big.tile([128, M], BF16, name="KT")          # [2h*64d, m]
        QT = big.tile([128, N], BF16, name="QT")          # [2h*64d, n]
        V_sb = big.tile([128, MB, 130], BF16, name="V")   # [m%128, mb, Vh0|1|Vh1|1]

        ones_f = consts.tile([128, MB], F32)
        nc.vector.memset(ones_f[:], 1.0)
        nc.vector.tensor_copy(V_sb[:, :, 64:65], ones_f[:])
        nc.vector.tensor_copy(V_sb[:, :, 129:130], ones_f[:])
        ones_s = consts.tile([65, 64], F32, name="ones_s")
        nc.vector.memset(ones_s[64:65, :], 1.0)
        nc.vector.tensor_copy(ones_w[64:65, :], ones_s[64:65, :])

        # ---------------- ctx phase: K/V/Q projections ----------------
        with (
            tc.tile_pool(name="ppsum", bufs=3, space="PSUM") as ppsum,
            tc.tile_pool(name="tpsum", bufs=2, space="PSUM") as tpsum,
        ):
            for ch in range(NT):
                sl = slice(ch * 1024, (ch + 1) * 1024)
                cblk = cin.tile([128, 8, 1024], BF16, tag="cin")
                nc.sync.dma_start(
                    out=cblk[:], in_=ct[:, sl].rearrange("(kb p) m -> p kb m", p=128)
                )
                xblk = xin.tile([128, 8, 1024], BF16, tag="xin")
                nc.sync.dma_start(
                    out=xblk[:], in_=xt[:, sl].rearrange("(kb p) n -> p kb n", p=128)
                )
                pk = ppsum.tile([128, 1024], F32, tag="pp")
                for kb in range(8):
                    for s in range(2):
                        nc.tensor.matmul(pk[:, s * 512:(s + 1) * 512],
                                         lhsT=wk_sb[:, kb, :],
                                         rhs=cblk[:, kb, s * 512:(s + 1) * 512],
                                         start=(kb == 0), stop=(kb == 7))
                nc.vector.tensor_copy(KT[:, sl], pk[:])
                pv = ppsum.tile([128, 1024], F32, tag="pp")
                for kb in range(8):
                    for s in range(2):
                        nc.tensor.matmul(pv[:, s * 512:(s + 1) * 512],
                                         lhsT=wv_sb[:, kb, :],
                                         rhs=cblk[:, kb, s * 512:(s + 1) * 512],
                                         start=(kb == 0), stop=(kb == 7))
                vts = vstage.tile([128, 1024], BF16, tag="vts")
                nc.vector.tensor_copy(vts[:], pv[:])
                tpv = tpsum.tile([128, 1024], BF16, tag="tp")
                for tb in range(8):
                    nc.tensor.transpose(
                        tpv[:, tb * 128:(tb + 1) * 128],
                        vts[:, tb * 128:(tb + 1) * 128],
                        identb[:],
                    )
                tv = tpv.rearrange("p (t d) -> p t d", t=8)
                nc.vector.tensor_copy(V_sb[:, ch * 8:(ch + 1) * 8, 0:64],
                                      tv[:, :, 0:64])
                nc.vector.tensor_copy(V_sb[:, ch * 8:(ch + 1) * 8, 65:129],
                                      tv[:, :, 64:128])
                pq = ppsum.tile([128, 1024], F32, tag="pp")
                for kb in range(8):
                    for s in range(2):
                        nc.tensor.matmul(pq[:, s * 512:(s + 1) * 512],
                                         lhsT=wq_sb[:, kb, :],
                                         rhs=xblk[:, kb, s * 512:(s + 1) * 512],
                                         start=(kb == 0), stop=(kb == 7))
                nc.vector.tensor_copy(QT[:, sl], pq[:])

        # ---------------- attention + out-proj, per n-chunk ----------------
        with (
            tc.tile_pool(name="spool", bufs=P, space="PSUM") as spool,
            tc.tile_pool(name="avpool", bufs=2, space="PSUM") as avpool,
        ):
            us = {}

            def s_exp(c, mb):
                """Both heads' scores for one m-block -> one wide exp."""
                ns = slice(c * NC, (c + 1) * NC)
                sp = spool.tile([128, 1024], F32, tag="sp", name="sp")
                for h in range(2):
                    nc.tensor.matmul(
                        sp[:, h * NC:(h + 1) * NC],
                        lhsT=KT[64 * h:64 * h + 64, mb * 128:(mb + 1) * 128],
                        rhs=QT[64 * h:64 * h + 64, ns],
                        start=True, stop=True,
                    )
                u = upool.tile([128, 1024], BF16, tag="u", name="u")
                nc.scalar.activation(u[:], sp[:], EXP, scale=SCALE)
                us[(c, mb)] = u

            def make_yp(c, nb, OTc):
                def step():
                    yp = spool.tile([128, 1024], F32, tag="sp", name="yp")
                    for s in range(2):
                        for h in range(2):
                            nc.tensor.matmul(
                                yp[:, s * 512:(s + 1) * 512],
                                lhsT=OTc[h][:, nb * 128:(nb + 1) * 128],
                                rhs=wo_sb[:, h, s * 512:(s + 1) * 512],
                                start=(h == 0), stop=(h == 1),
                            )
                    ys = ysb.tile([128, 1024], F32, tag="ys")
                    nc.vector.tensor_copy(ys[:], yp[:])
                    nc.sync.dma_start(
                        out=y[c * NC + nb * 128:c * NC + (nb + 1) * 128, :],
                        in_=ys[:],
                    )
                return step

            for mb in range(P):
                s_exp(0, mb)
            steps = []          # deferred (slot, emit_fn) from previous chunk
            for c in range(CH):
                av = [avpool.tile([65, NC], F32, tag="av", name=f"av{h}")
                      for h in range(2)]
                for mb in range(MB):
                    for h in range(2):
                        nc.tensor.matmul(
                            av[h][:],
                            lhsT=V_sb[:, mb, 65 * h:65 * h + 65],
                            rhs=us[(c, mb)][:, h * NC:(h + 1) * NC],
                            start=(mb == 0), stop=(mb == MB - 1),
                        )
                    del us[(c, mb)]
                    k = mb + P
                    if k < MB:
                        s_exp(c, k)
                    elif c + 1 < CH:
                        # next chunk's S/exp prologue, interleaved so ACT
                        # streams seamlessly across the chunk boundary
                        s_exp(c + 1, k - MB)
                    while steps and steps[0][0] <= mb:
                        steps.pop(0)[1]()
                assert not steps
                # normalize: reciprocal of the rowsum rows, then a ones[1,64]
                # matmul broadcasts 1/l across 64 partitions (psum); engines
                # read only ONE psum operand, so stage the broadcast in SBUF
                rr = [rrp.tile([65, NC], F32R, tag="rr", name=f"rr{h}")
                      for h in range(2)]
                with nc.allow_low_precision(reason="f32r == f32 bits"):
                    for h in range(2):
                        nc.vector.reciprocal(rr[h][64:65, :], av[h][64:65, :])
                bc = spool.tile([128, 1024], F32, tag="sp", name="bc")
                for h in range(2):
                    nc.tensor.matmul(bc[0:64, h * NC:(h + 1) * NC],
                                     lhsT=ones_w[64:65, :], rhs=rr[h][64:65, :],
                                     start=True, stop=True)
                bcs = rrp.tile([64, 2 * NC], F32, tag="bcs", name="bcs")
                nc.vector.tensor_copy(bcs[:], bc[0:64, :])
                OTc = [otp.tile([64, NC], BF16, tag="ot", name=f"ot{h}")
                       for h in range(2)]
                for h in range(2):
                    nc.vector.tensor_mul(OTc[h][:], av[h][0:64, :],
                                         bcs[:, h * NC:(h + 1) * NC])
                # out-proj: spread into the next chunk's AV loop (PE has
                # ~185ns/mb of slack there); flush immediately on last chunk
                new_steps = [(2 + 2 * nb, make_yp(c, nb, OTc)) for nb in range(4)]
                if c + 1 < CH:
                    steps = new_steps
                else:
                    for _, fn in new_steps:
                        fn()


# ---------------------------------------------------------------------------
_NC_CACHE = {}


def _get_nc():
    if "full" not in _NC_CACHE:
        _NC_CACHE["full"] = build(4096, 4096, 512)
    return _NC_CACHE["full"]


def make_in_maps(x, context, Wq, Wk, Wv, Wo, bo):
    import ml_dtypes
    bf16 = ml_dtypes.bfloat16
    x = np.asarray(x, dtype=np.float32)
    context = np.asarray(context, dtype=np.float32)
    Wq = np.asarray(Wq, dtype=np.float32).astype(bf16)
    Wk = np.asarray(Wk, dtype=np.float32).astype(bf16)
    Wv = np.asarray(Wv, dtype=np.float32).astype(bf16)
    WoB = np.asarray(Wo, dtype=np.float32).astype(bf16)
    xT = [np.ascontiguousarray(x[b].T).astype(bf16) for b in range(x.shape[0])]
    cT = [np.ascontiguousarray(context[b].T).astype(bf16)
          for b in range(context.shape[0])]
    in_maps = []
    for core in range(N_CORES):
        b, g = core // 4, core % 4
        sl = slice(g * DG, (g + 1) * DG)
        in_maps.append({
            "xt": xT[b],
            "ct": cT[b],
            "wq": np.ascontiguousarray(Wq[:, sl]),
            "wk": np.ascontiguousarray(Wk[:, sl]),
            "wv": np.ascontiguousarray(Wv[:, sl]),
            "wo": np.ascontiguousarray(WoB[sl, :]),
        })
    return in_maps


def combine(results, bo):
    out = np.empty((2, 4096, 1024), np.float32)
    for b in range(2):
        acc = results[4 * b]["y"].copy()
        for g in range(1, 4):
            acc += results[4 * b + g]["y"]
        out[b] = acc + bo
    return out


def kernel(x, context, Wq, Wk, Wv, Wo, bo):
    nc = _get_nc()
    bo = np.asarray(bo, dtype=np.float32)
    in_maps = make_in_maps(x, context, Wq, Wk, Wv, Wo, bo)
    res = run_bass_kernel_spmd(nc, in_maps, list(range(N_CORES))).results
    return combine(res, bo)
